# revision 1
# baseline (speedup 1.0000x reference)
"""Trainium2 Bass kernel for nn_LocalTrans (gnn message passing).

Math (reference, with exact simplifications):
  k = f@kw + kb ; v = f@vw + vb            (per batch cloud)
  kg, vg = gather(k, idx), gather(v, idx)  [B,N,K,C]
  attn = softmax((q - kg)/8, axis=K)       == softmax(-kg/8)  (q, kb const over K)
  ctx  = max_k (attn - 1) * vg
       = (1/s) * max_k (e_k - s) * vg_k,   e = exp(-kg/8), s = sum_k e
  h    = ctx@fw + fb ; BatchNorm(global mean/var over B*N) ; LeakyReLU(0.2)
  out  = f + h

Sharding: 8 cores; core c -> batch c//2, node-half c%2 (8192 nodes).
Each core projects k,v for its full batch into a packed fp16 kv table
[16384, 128] in DRAM (rows = [k(64)|v(64)], 256B), then dma_gathers the
16 neighbor rows per node. BN stats are AllReduced across cores.
"""

import sys

if "/opt/trn_rl_repo" not in sys.path:
    sys.path.insert(0, "/opt/trn_rl_repo")

import numpy as np
from contextlib import ExitStack

import concourse.bass as bass
import concourse.bacc as bacc
import concourse.tile as tile
from concourse import mybir
from concourse.bass_utils import run_bass_kernel_spmd

F32 = mybir.dt.float32
F16 = mybir.dt.float16
I16 = mybir.dt.int16
AF = mybir.ActivationFunctionType
OP = mybir.AluOpType

B, N, C, K = 4, 16384, 64, 16
N_CORES = 8
NODES = N // 2            # nodes per core
TILES = NODES // 128      # 64 local node tiles
PTILES = N // 128         # 128 projection tiles (full batch)
GTILES = 2                # node tiles per gather group
GROUPS = TILES // GTILES  # 32
GIDX = GTILES * 128 * K   # 4096 indices per gather
EPS = 1e-5
ALPHA = 0.2
M_TOT = float(B * N)
KV_CH = 2 * C             # 128: packed [k|v] channels

def _build_program(n=N, n_cores=N_CORES, sim=False, do_collective=True, do_phase3=True, do_phase2=True, max_groups=None, p2_ops=99, do_phase1=True, do_consts=True):
    # derived sizes (n = batch cloud size); defaults build the real program
    nodes = n // 2
    tiles = nodes // 128
    ptiles = n // 128
    groups = tiles // GTILES
    gidx = GTILES * 128 * K
    pchunk = min(16, ptiles)
    ochunk = min(16, tiles)
    m_tot = float(B * n) if not sim else float(n_cores * nodes)
    nc = bacc.Bacc(None)

    feat_in = nc.dram_tensor("feat", [128, ptiles * C], F32, kind="ExternalInput")
    idxw_in = nc.dram_tensor("idxw", [128, tiles * K * 8], I16, kind="ExternalInput")
    kvwb_in = nc.dram_tensor("kvwb", [C + 1, KV_CH], F32, kind="ExternalInput")
    fw_in = nc.dram_tensor("fw", [C, C], F32, kind="ExternalInput")
    fbc_in = nc.dram_tensor("fbc", [C, 1], F32, kind="ExternalInput")
    gammac_in = nc.dram_tensor("gammac", [C, 1], F32, kind="ExternalInput")
    betac_in = nc.dram_tensor("betac", [C, 1], F32, kind="ExternalInput")
    ident_in = nc.dram_tensor("ident", [128, 128], F32, kind="ExternalInput")
    onescol_in = nc.dram_tensor("onescol", [128, 1], F32, kind="ExternalInput")
    onesrow_in = nc.dram_tensor("onesrow", [1, tiles * 128], F32, kind="ExternalInput")

    out_dram = nc.dram_tensor("out", [128, tiles * C], F32, kind="ExternalOutput")

    kv_dram = nc.dram_tensor("kv_table", [n, KV_CH], F16)
    cc_in = nc.dram_tensor("cc_in", [C, C + 1], F32)
    cc_out = nc.dram_tensor("cc_out", [C, C + 1], F32, addr_space="Shared")

    with tile.TileContext(nc) as tc:
        with ExitStack() as ctx:
            cpool = ctx.enter_context(tc.tile_pool(name="const", bufs=1))
            wpool = ctx.enter_context(tc.tile_pool(name="work", bufs=2))
            gpool = ctx.enter_context(tc.tile_pool(name="gath", bufs=3))
            pspool = ctx.enter_context(tc.tile_pool(name="ps", bufs=2, space="PSUM"))
            accpool = ctx.enter_context(tc.tile_pool(name="acc", bufs=1, space="PSUM"))

            # ---- resident inputs ----
            idxw_sb = cpool.tile([128, tiles * K * 8], I16)
            nc.sync.dma_start(idxw_sb[:], idxw_in[:])
            feat_sb = cpool.tile([128, ptiles * C], F32)
            if do_consts:
                nc.sync.dma_start(feat_sb[:], feat_in[:])
            kvwb_sb = cpool.tile([C + 1, KV_CH], F32)
            if do_consts:
                nc.sync.dma_start(kvwb_sb[:], kvwb_in[:])
            fw_sb = cpool.tile([C, C], F32)
            if do_consts:
                nc.sync.dma_start(fw_sb[:], fw_in[:])
            fbc_sb = cpool.tile([C, 1], F32)
            if do_consts:
                nc.sync.dma_start(fbc_sb[:], fbc_in[:])
            gammac_sb = cpool.tile([C, 1], F32)
            if do_consts:
                nc.sync.dma_start(gammac_sb[:], gammac_in[:])
            betac_sb = cpool.tile([C, 1], F32)
            if do_consts:
                nc.sync.dma_start(betac_sb[:], betac_in[:])
            ident_sb = cpool.tile([128, 128], F32)
            if do_consts:
                nc.sync.dma_start(ident_sb[:], ident_in[:])
            onescol_sb = cpool.tile([128, 1], F32)
            if do_consts:
                nc.sync.dma_start(onescol_sb[:], onescol_in[:])

            # ctxT: [ctx^T; ones] for phase-3 matmul lhsT, ones row loaded once
            ctxT_sb = cpool.tile([C + 1, tiles * 128], F32)
            if do_consts:
                nc.sync.dma_start(ctxT_sb[C : C + 1, :], onesrow_in[:])

            # ---- phase 1: kv projection for the full batch ----
            kv_view = kv_dram[:, :].rearrange("(t p) c -> p t c", p=128)
            for t in range(ptiles if do_phase1 else 0):
                ft_ps = pspool.tile([C, 128], F32, tag="mm")
                nc.tensor.transpose(
                    ft_ps[:], feat_sb[:, t * C : (t + 1) * C], ident_sb[:]
                )
                ft65 = wpool.tile([C + 1, 128], F32, tag="ft65")
                nc.scalar.copy(ft65[0:C, :], ft_ps[:])
                nc.gpsimd.memset(ft65[C : C + 1, :], 1.0)
                kv_ps = pspool.tile([128, KV_CH], F32, tag="kvps")
                nc.tensor.matmul(kv_ps[:], lhsT=ft65[:], rhs=kvwb_sb[:])
                if t % pchunk == 0:
                    kv_sb = wpool.tile([128, pchunk * KV_CH], F16, tag="kvsb")
                nc.vector.tensor_copy(
                    kv_sb[:, (t % pchunk) * KV_CH : (t % pchunk + 1) * KV_CH], kv_ps[:]
                )
                if t % pchunk == pchunk - 1:
                    nc.sync.dma_start(
                        kv_view[:, t - pchunk + 1 : t + 1, :], kv_sb[:]
                    )

            # ---- phase 2: gather + attention + BN stats ----
            use_acc = do_phase2 and p2_ops >= 6
            if use_acc:
                gram_ps = accpool.tile([C, C], F32, tag="gram")
                sum_ps = accpool.tile([C, 1], F32, tag="sum")

            ngroups = (groups if do_phase2 else 0) if max_groups is None else min(groups, max_groups)
            for g in range(ngroups):
                G = gpool.tile([128, GTILES * K, KV_CH], F16, tag="G")
                nc.gpsimd.dma_gather(
                    G[:],
                    kv_dram[:, :],
                    idxw_sb[:, g * (gidx // 16) : (g + 1) * (gidx // 16)],
                    gidx,
                    gidx,
                    KV_CH,
                    single_packet=False,
                )
                if p2_ops < 1:
                    continue
                E = wpool.tile([128, GTILES, K, C], F16, tag="E")
                nc.scalar.activation(
                    E[:],
                    G[:, :, 0:C].rearrange("p (a b) c -> p a b c", a=GTILES),
                    AF.Exp,
                    scale=-0.125,
                )
                if p2_ops < 2:
                    continue
                T1 = wpool.tile([128, GTILES, K // 2, C], F16, tag="T1")
                nc.vector.tensor_add(T1[:], E[:, :, 0:8, :], E[:, :, 8:16, :])
                T2 = wpool.tile([128, GTILES, K // 4, C], F16, tag="T2")
                nc.vector.tensor_add(T2[:], T1[:, :, 0:4, :], T1[:, :, 4:8, :])
                T3 = wpool.tile([128, GTILES, K // 8, C], F16, tag="T3")
                nc.vector.tensor_add(T3[:], T2[:, :, 0:2, :], T2[:, :, 2:4, :])
                s16 = wpool.tile([128, GTILES, C], F16, tag="s16")
                nc.vector.tensor_add(s16[:], T3[:, :, 0, :], T3[:, :, 1, :])

                if p2_ops < 3:
                    continue
                r32 = wpool.tile([128, GTILES, C], F32, tag="r32")
                nc.vector.reciprocal(r32[:], s16[:])

                s_b = s16[:].rearrange("p a (b c) -> p a b c", b=1).broadcast_to(
                    [128, GTILES, K, C]
                )
                D = wpool.tile([128, GTILES, K, C], F16, tag="D")
                nc.vector.tensor_sub(D[:], E[:], s_b)
                Gv = G[:, :, C:KV_CH].rearrange("p (a b) c -> p a b c", a=GTILES)
                W = wpool.tile([128, GTILES, K, C], F16, tag="W")
                nc.vector.tensor_mul(W[:], D[:], Gv)

                if p2_ops < 4:
                    continue
                M1 = wpool.tile([128, GTILES, K // 2, C], F16, tag="M1")
                nc.vector.tensor_tensor(M1[:], W[:, :, 0:8, :], W[:, :, 8:16, :], OP.max)
                M2 = wpool.tile([128, GTILES, K // 4, C], F16, tag="M2")
                nc.vector.tensor_tensor(M2[:], M1[:, :, 0:4, :], M1[:, :, 4:8, :], OP.max)
                M3 = wpool.tile([128, GTILES, K // 8, C], F16, tag="M3")
                nc.vector.tensor_tensor(M3[:], M2[:, :, 0:2, :], M2[:, :, 2:4, :], OP.max)
                mx = wpool.tile([128, GTILES, C], F16, tag="mx")
                nc.vector.tensor_tensor(mx[:], M3[:, :, 0, :], M3[:, :, 1, :], OP.max)

                if p2_ops < 5:
                    continue
                ctx32 = wpool.tile([128, GTILES, C], F32, tag="ctx32")
                nc.vector.tensor_mul(ctx32[:], mx[:], r32[:])

                if p2_ops < 6:
                    continue
                for tl in range(GTILES):
                    t = g * GTILES + tl
                    ct_ps = pspool.tile([C, 128], F32, tag="mm")
                    nc.tensor.transpose(ct_ps[:], ctx32[:, tl, :], ident_sb[:])
                    nc.scalar.copy(ctxT_sb[0:C, t * 128 : (t + 1) * 128], ct_ps[:])
                    nc.tensor.matmul(
                        gram_ps[:],
                        lhsT=ctx32[:, tl, :],
                        rhs=ctx32[:, tl, :],
                        start=(t == 0),
                        stop=(t == 2 * ngroups - 1),
                        skip_group_check=True,
                    )
                    nc.tensor.matmul(
                        sum_ps[:],
                        lhsT=ctx32[:, tl, :],
                        rhs=onescol_sb[:],
                        start=(t == 0),
                        stop=(t == 2 * ngroups - 1),
                        skip_group_check=True,
                    )

            # ---- BN stats allreduce + fold into weights ----
            stat_sb = cpool.tile([C, C + 1], F32)
            if use_acc:
                nc.vector.tensor_copy(stat_sb[:, 0:C], gram_ps[:])
                nc.vector.tensor_copy(stat_sb[:, C : C + 1], sum_ps[:])
            else:
                nc.gpsimd.memset(stat_sb[:], 1.0)
            if do_collective:
                nc.sync.dma_start(cc_in[:], stat_sb[:])
                nc.gpsimd.collective_compute(
                    "AllReduce",
                    OP.add,
                    replica_groups=[list(range(n_cores))],
                    ins=[cc_in[:]],
                    outs=[cc_out[:]],
                )
                stat2_sb = cpool.tile([C, C + 1], F32)
                nc.sync.dma_start(stat2_sb[:], cc_out[:])
            else:
                stat2_sb = stat_sb

            if do_phase3:
                # u = fw^T sum_ctx / M ; var = diag(fw^T S2 fw)/M - u^2
                ps1 = pspool.tile([C, 1], F32, tag="mm")
                nc.tensor.matmul(ps1[:], lhsT=fw_sb[:], rhs=stat2_sb[:, C : C + 1])
                u_sb = cpool.tile([C, 1], F32)
                nc.scalar.copy(u_sb[:], ps1[:])
                nc.scalar.mul(u_sb[:], u_sb[:], 1.0 / m_tot)
                psT = pspool.tile([C, C], F32, tag="mm")
                nc.tensor.matmul(psT[:], lhsT=stat2_sb[:, 0:C], rhs=fw_sb[:])
                T_sb = cpool.tile([C, C], F32)
                nc.scalar.copy(T_sb[:], psT[:])
                P_sb = cpool.tile([C, C], F32)
                nc.vector.tensor_mul(P_sb[:], T_sb[:], fw_sb[:])
                psE2 = pspool.tile([C, 1], F32, tag="mm")
                nc.tensor.matmul(psE2[:], lhsT=P_sb[:], rhs=onescol_sb[0:C, :])
                u2_sb = cpool.tile([C, 1], F32)
                nc.scalar.activation(u2_sb[:], u_sb[:], AF.Square)
                var_sb = cpool.tile([C, 1], F32)
                nc.scalar.copy(var_sb[:], psE2[:])
                nc.scalar.mul(var_sb[:], var_sb[:], 1.0 / m_tot)
                nc.vector.tensor_sub(var_sb[:], var_sb[:], u2_sb[:])
                eps_sb = cpool.tile([C, 1], F32)
                nc.gpsimd.memset(eps_sb[:], EPS)
                sq_sb = cpool.tile([C, 1], F32)
                nc.scalar.activation(sq_sb[:], var_sb[:], AF.Sqrt, bias=eps_sb[:])
                rsq_sb = cpool.tile([C, 1], F32)
                nc.vector.reciprocal(rsq_sb[:], sq_sb[:])
                scale_sb = cpool.tile([C, 1], F32)
                nc.vector.tensor_mul(scale_sb[:], rsq_sb[:], gammac_sb[:])
                mean_sb = cpool.tile([C, 1], F32)
                nc.vector.tensor_add(mean_sb[:], u_sb[:], fbc_sb[:])
                # bprime = fb*scale + (beta - mean*scale) = (fb - mean)*scale + beta
                bp_sb = cpool.tile([C, 1], F32)
                nc.vector.tensor_sub(bp_sb[:], fbc_sb[:], mean_sb[:])
                nc.vector.tensor_mul(bp_sb[:], bp_sb[:], scale_sb[:])
                nc.vector.tensor_add(bp_sb[:], bp_sb[:], betac_sb[:])

                # rows: scale_row/bp_row via PE transpose, then bcast scale to 64 rows
                ps_sr = pspool.tile([1, C], F32, tag="mm")
                nc.tensor.transpose(ps_sr[:], scale_sb[:], ident_sb[0:C, 0:C])
                sr_sb = cpool.tile([1, C], F32)
                nc.scalar.copy(sr_sb[:], ps_sr[:])
                ps_bp = pspool.tile([1, C], F32, tag="mm")
                nc.tensor.transpose(ps_bp[:], bp_sb[:], ident_sb[0:C, 0:C])
                bpr_sb = cpool.tile([1, C], F32)
                nc.scalar.copy(bpr_sb[:], ps_bp[:])
                ones1_sb = cpool.tile([1, C], F32)
                nc.gpsimd.memset(ones1_sb[:], 1.0)
                ps_b64 = pspool.tile([C, C], F32, tag="mm")
                nc.tensor.matmul(ps_b64[:], lhsT=ones1_sb[:], rhs=sr_sb[:])
                scale64_sb = cpool.tile([C, C], F32)
                nc.scalar.copy(scale64_sb[:], ps_b64[:])
                rhs2_sb = cpool.tile([C + 1, C], F32)
                nc.vector.tensor_mul(rhs2_sb[0:C, :], fw_sb[:], scale64_sb[:])
                nc.scalar.copy(rhs2_sb[C : C + 1, :], bpr_sb[:])

                # ---- phase 3: h_bn = ctx@fw' + b', lrelu, residual, store ----
                for t in range(tiles):
                    h_ps = pspool.tile([128, C], F32, tag="mm")
                    nc.tensor.matmul(
                        h_ps[:],
                        lhsT=ctxT_sb[:, t * 128 : (t + 1) * 128],
                        rhs=rhs2_sb[:],
                    )
                    h_sb = wpool.tile([128, C], F32, tag="hsb")
                    nc.scalar.copy(h_sb[:], h_ps[:])
                    lr_sb = wpool.tile([128, C], F32, tag="lr")
                    nc.vector.scalar_tensor_tensor(
                        lr_sb[:], h_sb[:], ALPHA, h_sb[:], OP.mult, OP.max
                    )
                    if t % ochunk == 0:
                        out_sb = wpool.tile([128, ochunk * C], F32, tag="outsb")
                    nc.gpsimd.tensor_add(
                        out_sb[:, (t % ochunk) * C : (t % ochunk + 1) * C],
                        lr_sb[:],
                        feat_sb[:, t * C : (t + 1) * C],
                    )
                    if t % ochunk == ochunk - 1:
                        nc.sync.dma_start(
                            out_dram[:, (t - ochunk + 1) * C : (t + 1) * C], out_sb[:]
                        )

    nc.compile()
    return nc


_PROG = None


def _get_program():
    global _PROG
    if _PROG is None:
        _PROG = _build_program()
    return _PROG


def _prep_core_inputs(features, idx, kvwb, fw, fbc, gammac, betac, ident, onescol,
                      onesrow, core):
    b, h = core // 2, core % 2
    # own half first so local node ids are SPMD-uniform
    f = np.roll(np.asarray(features[b]), -h * NODES, axis=0)
    feat_t = np.ascontiguousarray(
        f.reshape(PTILES, 128, C).transpose(1, 0, 2).reshape(128, PTILES * C)
    )
    idx_loc = (np.asarray(idx[b, h * NODES : (h + 1) * NODES]).astype(np.int64)
               - h * NODES) % N
    # gather flat order per group: slot = tl*K + j, flat[slot*128 + p]
    a = idx_loc.reshape(GROUPS, GTILES, 128, K).transpose(0, 1, 3, 2)
    flat = a.reshape(GROUPS, GIDX)
    w = flat.reshape(GROUPS, GIDX // 16, 16).transpose(0, 2, 1)  # g, 16, cols
    idxw = np.ascontiguousarray(
        w.transpose(1, 0, 2).reshape(16, GROUPS * (GIDX // 16))
    ).astype(np.int16)
    idxw = np.tile(idxw, (8, 1))
    return {
        "feat": feat_t,
        "idxw": idxw,
        "kvwb": kvwb,
        "fw": fw,
        "fbc": fbc,
        "gammac": gammac,
        "betac": betac,
        "ident": ident,
        "onescol": onescol,
        "onesrow": onesrow,
    }


def kernel(features, pos, qw, qb, kw, kb, vw, vb, fw, fb, gamma, beta, idx):
    del pos, qw, qb  # do not affect the output (constant over the softmax axis)
    nc = _get_program()

    features = np.asarray(features, np.float32)
    kvwb = np.concatenate(
        [
            np.concatenate([np.asarray(kw), np.asarray(vw)], axis=1),
            np.concatenate([np.asarray(kb), np.asarray(vb)])[None, :],
        ],
        axis=0,
    ).astype(np.float32)
    fw32 = np.asarray(fw, np.float32)
    fbc = np.asarray(fb, np.float32).reshape(C, 1)
    gammac = np.asarray(gamma, np.float32).reshape(C, 1)
    betac = np.asarray(beta, np.float32).reshape(C, 1)
    ident = np.eye(128, dtype=np.float32)
    onescol = np.ones((128, 1), np.float32)
    onesrow = np.ones((1, TILES * 128), np.float32)
    idx = np.asarray(idx)

    in_maps = [
        _prep_core_inputs(features, idx, kvwb, fw32, fbc, gammac, betac, ident,
                          onescol, onesrow, c)
        for c in range(N_CORES)
    ]
    res = run_bass_kernel_spmd(nc, in_maps, list(range(N_CORES)))

    out = np.empty((B, N, C), np.float32)
    for c in range(N_CORES):
        b, h = c // 2, c % 2
        o = res.results[c]["out"]  # [128, TILES*C]
        o = o.reshape(128, TILES, C).transpose(1, 0, 2).reshape(NODES, C)
        out[b, h * NODES : (h + 1) * NODES] = o
    return out

